# revision 1
# baseline (speedup 1.0000x reference)
"""EquivariantAttention Trainium2 kernel.

B=8 batches data-parallel over 8 NeuronCores; per core:
  qkv = x @ W_qkv + b_qkv ; dist = cdist(g, g)
  S^T[j,i] = (q_i.k_j)/sqrt(H) * exp(-dist)   (computed transposed: j on partitions)
  U^T = exp(S^T)  (no max-subtraction; values bounded)
  out^T[h,i] = V^T @ U^T, normalized by row-sums l_i (ones-matmul on PE)
  y = out @ W_out + b_out

Matmuls run as float32r (full-rate on PE at moving-dim >= 256); attention
weights U^T and E=exp(-dist) are bf16 (dist itself is f32). d2 is computed
on PE with augmented features [g, |g|^2, 1] . [-2g, 1, |g|^2] packed into
one SBUF tile at partition offsets 0 and 32.

ACT table sets: sqrt and exp live in different sets and a table load costs
~2.7us, so per i-chunk the ACT work is batched [sqrt x16][exp x16][expU x16]
-> 2 loads per chunk. All copies/elementwise ops are pinned to DVE.
"""

import numpy as np

import concourse.bass as bass
from concourse import bacc
import concourse.mybir as mybir
import concourse.tile as tile
from concourse.masks import make_identity
from concourse.tile import add_dep_helper

P = 128
H = 512
SC = 512
HT = H // P  # 4

f32 = mybir.dt.float32
f32r = mybir.dt.float32r
bf16 = mybir.dt.bfloat16
AF = mybir.ActivationFunctionType
OP = mybir.AluOpType
RSQRT_H = 1.0 / float(np.sqrt(H))


def _body(tc, n, x, g, wqkv, bqkv, wout, bout, y):
    nc = tc.nc
    NT = n // P
    NC_ = n // SC
    ITC = SC // P  # i-tiles per chunk (4)

    with (
        nc.allow_low_precision(
            reason="float32r tiles feed FP32r matmuls; storage is fp32-width"
        ),
        tc.tile_pool(name="const", bufs=1) as const,
        tc.tile_pool(name="geo", bufs=1) as geo,
        tc.tile_pool(name="et_pool", bufs=2) as et_pool,
        tc.tile_pool(name="small", bufs=2) as small,
        tc.tile_pool(name="ps_s", bufs=2, space="PSUM") as ps_s,
        tc.tile_pool(name="ps_d", bufs=2, space="PSUM") as ps_d,
        tc.tile_pool(name="ps_o", bufs=3, space="PSUM") as ps_o,
        tc.tile_pool(name="ps_l", bufs=1, space="PSUM") as ps_l,
    ):
        # ---- persistent weights / constants ----
        wout_sb = const.tile([P, HT, H], f32r)
        nc.sync.dma_start(
            wout_sb, wout.rearrange("(kt p) m -> p kt m", p=P).bitcast(f32r)
        )
        bqk_sb = const.tile([P, 8], f32)  # cols 0-3: b_q m-tiles, 4-7: b_k
        nc.sync.dma_start(bqk_sb, bqkv[0 : 2 * H].rearrange("(mt p) -> p mt", p=P))
        # pre-scale q biases so the ACT Identity copyback computes (ps + b)*s
        # as ps*s + b*s with bias AP = b*s
        nc.vector.tensor_scalar_mul(bqk_sb[:, 0:4], bqk_sb[:, 0:4], RSQRT_H)
        bo_bc = const.tile([P, H], f32)
        nc.gpsimd.dma_start(bo_bc, bout.partition_broadcast(P))
        ones_bf = const.tile([P, 1], bf16)
        nc.vector.memset(ones_bf, 1.0)
        ones_row = const.tile([1, P], f32r)
        nc.vector.memset(ones_row.bitcast(f32), 1.0)

        # augmented geometry, transposed: d2[j,i] = sum_k h_k[j] * g_k[i]
        hT8 = geo.tile([8, n], f32r)
        gT8 = geo.tile([8, n], f32r)

        # ---- q/k/v projection ----
        with tc.tile_pool(name="qkv", bufs=1) as qkv:
            qT = qkv.tile([P, HT, n], f32r)  # q^T / sqrt(H), [h, i]
            kT = qkv.tile([P, HT, n], f32r)  # k^T, [h, j]
            v_bf = qkv.tile([P, NT, H], bf16)  # v natural, [j, h]

            with tc.tile_pool(name="xt_pool", bufs=1) as xt_pool:
                xT = xt_pool.tile([P, HT, n], f32r)
                ident = xt_pool.tile([P, P], f32)
                make_identity(nc, ident)

                with tc.tile_pool(name="wstage", bufs=1) as wstage:
                    # weight DMA first so it overlaps the x transposes
                    wqkv_sb = wstage.tile([P, HT, 3 * H], f32r)
                    nc.gpsimd.dma_start(
                        wqkv_sb,
                        wqkv.rearrange("(kt p) m -> p kt m", p=P).bitcast(f32r),
                    )
                    bv_bc = wstage.tile([P, H], f32)
                    nc.gpsimd.dma_start(
                        bv_bc, bqkv[2 * H : 3 * H].partition_broadcast(P)
                    )
                    g_sb = wstage.tile([P, NT, 3], f32)
                    nc.sync.dma_start(g_sb, g.rearrange("(nt p) c -> p nt c", p=P))
                    g2 = wstage.tile([P, NT, 3], f32)
                    nc.vector.tensor_mul(g2, g_sb, g_sb)
                    sq = wstage.tile([P, NT, 1], f32)
                    nc.vector.reduce_sum(sq, g2, axis=mybir.AxisListType.X)
                    Ag = wstage.tile([P, NT, 8], f32)  # [g, |g|^2, 1, 0..]
                    Ah = wstage.tile([P, NT, 8], f32)  # [-2g, 1, |g|^2, 0..]
                    nc.vector.memset(Ag, 0.0)
                    nc.vector.memset(Ah, 0.0)
                    nc.vector.tensor_copy(Ag[:, :, 0:3], g_sb)
                    nc.vector.tensor_copy(Ag[:, :, 3:4], sq)
                    nc.vector.memset(Ag[:, :, 4:5], 1.0)
                    nc.vector.tensor_scalar_mul(Ah[:, :, 0:3], g_sb, -2.0)
                    nc.vector.memset(Ah[:, :, 3:4], 1.0)
                    nc.vector.tensor_copy(Ah[:, :, 4:5], sq)
                    for nt in range(NT):
                        pt = ps_s.tile([P, SC], f32, tag="pss")
                        nc.tensor.transpose(pt[:8, :P], Ah[:, nt, :], ident)
                        nc.scalar.copy(hT8[:, nt * P : (nt + 1) * P], pt[:8, :P])
                        pt2 = ps_d.tile([P, SC], f32, tag="psd")
                        nc.tensor.transpose(pt2[:8, :P], Ag[:, nt, :], ident)
                        nc.scalar.copy(gT8[:, nt * P : (nt + 1) * P], pt2[:8, :P])
                    NH = max(1, NT // 4)
                    x_r = x.rearrange("(nt p) h -> p nt h", p=P)
                    with tc.tile_pool(name="xsb_pool", bufs=2) as xsb_pool:
                        for qi, hh in enumerate(range(0, NT, NH)):
                            x_sb = xsb_pool.tile([P, NH, H], f32, tag="x_sb")
                            eng = (nc.sync, nc.scalar)[qi % 2]
                            eng.dma_start(x_sb, x_r[:, hh : hh + NH, :])
                            for nt in range(NH):
                                for ht in range(HT):
                                    tp_pool, tp_tag = (
                                        (ps_o, "pso"),
                                        (ps_s, "pss"),
                                        (ps_d, "psd"),
                                    )[(nt * HT + ht) % 3]
                                    pt = tp_pool.tile([P, SC], f32, tag=tp_tag)
                                    nc.tensor.transpose(
                                        pt[:, :P],
                                        x_sb[:, nt, ht * P : (ht + 1) * P],
                                        ident,
                                    )
                                    dst_ap = xT[
                                        :, ht, (hh + nt) * P : (hh + nt + 1) * P
                                    ]
                                    if ht % 2 == 0:
                                        nc.scalar.copy(dst_ap, pt[:, :P])
                                    else:
                                        nc.vector.tensor_copy(dst_ap, pt[:, :P])

                    # q^T and k^T tiles; m-tile mt in 0..8 (0-3 q, 4-7 k)
                    for mt in range(8):
                        dst = qT if mt < 4 else kT
                        mi = mt % 4
                        scale = RSQRT_H if mt < 4 else 1.0
                        for c in range(NC_):
                            ps = ps_s.tile([P, SC], f32, tag="pss")
                            for kc in range(HT):
                                nc.tensor.matmul(
                                    ps,
                                    lhsT=wqkv_sb[:, kc, mt * P : (mt + 1) * P],
                                    rhs=xT[:, kc, c * SC : (c + 1) * SC],
                                    start=(kc == 0),
                                    stop=(kc == HT - 1),
                                )
                            # out = ps*scale + (b*scale) on ACT (idle in prologue)
                            nc.scalar.activation(
                                dst[:, mi, c * SC : (c + 1) * SC],
                                ps,
                                AF.Identity,
                                bias=bqk_sb[:, mt : mt + 1],
                                scale=scale,
                            )

                    # v natural: [j, h] (bf16), bias added
                    for nt in range(NT):
                        ps = ps_s.tile([P, SC], f32, tag="pss")
                        for kc in range(HT):
                            nc.tensor.matmul(
                                ps,
                                lhsT=xT[:, kc, nt * P : (nt + 1) * P],
                                rhs=wqkv_sb[:, kc, 2 * H : 3 * H],
                                start=(kc == 0),
                                stop=(kc == HT - 1),
                            )
                        nc.vector.tensor_add(v_bf[:, nt, :], ps, bv_bc)

            # ---- attention, transposed, software-pipelined over i-chunks ----
            with (
                tc.tile_pool(name="dist_pool", bufs=1) as dist_pool,
                tc.tile_pool(name="e_pool", bufs=1) as e_pool,
                tc.tile_pool(name="ut_pool", bufs=2) as ut_pool,
                tc.tile_pool(name="ot_pool", bufs=1) as ot_pool,
            ):
                y_r = y.rearrange("(nt p) h -> p nt h", p=P)
                state = {"prev": None}

                def chain(a):
                    # keep ACT in emission order so sqrt/exp table-set
                    # switches happen once per block, not per op
                    if state["prev"] is not None:
                        add_dep_helper(
                            a.ins,
                            state["prev"].ins,
                            sync=False,
                            reason="ACT table-set batching",
                        )
                    state["prev"] = a
                    return a

                dists = {}
                Es = {}

                def emit_sqrt(ic, jts=None):
                    # d2 on PE + clamp + sqrt block for chunk ic
                    isl = slice(ic * SC, (ic + 1) * SC)
                    if ic in dists:
                        dist = dists[ic]
                    else:
                        dist = dist_pool.tile([P, NT, SC], f32, tag="dist")
                        dists[ic] = dist
                    jl = list(jts) if jts is not None else list(range(NT))
                    for jt in jl:
                        psd = ps_d.tile([P, SC], f32, tag="psd")
                        nc.tensor.matmul(
                            psd,
                            lhsT=hT8[:, jt * P : (jt + 1) * P],
                            rhs=gT8[:, isl],
                            start=True,
                            stop=True,
                        )
                        # clamp writes PSUM->SBUF so the psd bank frees here,
                        # and sqrt can run in-place on SBUF pairs (1024-wide)
                        nc.vector.tensor_scalar_max(dist[:, jt, :], psd, 0.0)
                        if jt % 2 == 1 or jt == jl[-1]:
                            j0 = jt - (jt % 2)
                            chain(
                                nc.scalar.activation(
                                    dist[:, j0 : jt + 1, :],
                                    dist[:, j0 : jt + 1, :],
                                    AF.Sqrt,
                                )
                            )

                def emit_exp(ic):
                    # E = exp(-dist) block (pairs of j-tiles per ACT op)
                    dist = dists.pop(ic)
                    E = e_pool.tile([P, NT, SC], bf16, tag="E")
                    step = 4 if NT % 4 == 0 else 1
                    for jp in range(0, NT, step):
                        chain(
                            nc.scalar.activation(
                                E[:, jp : jp + step, :],
                                dist[:, jp : jp + step, :],
                                AF.Exp,
                                scale=-1.0,
                            )
                        )
                    Es[ic] = E

                emit_sqrt(0)
                emit_exp(0)
                for ic in range(NC_):
                    isl = slice(ic * SC, (ic + 1) * SC)
                    E = Es.pop(ic)
                    UT = ut_pool.tile([P, NT, SC], bf16, tag="UT")
                    psl = ps_l.tile([1, SC], f32, tag="psl")
                    for jt in range(NT):
                        jsl = slice(jt * P, (jt + 1) * P)
                        pss = ps_s.tile([P, SC], f32, tag="pss")
                        for kc in range(HT):
                            nc.tensor.matmul(
                                pss,
                                lhsT=kT[:, kc, jsl],
                                rhs=qT[:, kc, isl],
                                start=(kc == 0),
                                stop=(kc == HT - 1),
                            )
                        if jt % 2 == 0:
                            et2 = et_pool.tile([P, 2, SC], f32, tag="et")
                        nc.vector.tensor_mul(et2[:, jt % 2, :], pss, E[:, jt, :])
                        if jt % 2 == 1:
                            # exp + row-sum matmul over the pair of j-tiles
                            chain(
                                nc.scalar.activation(
                                    UT[:, jt - 1 : jt + 1, :], et2, AF.Exp
                                )
                            )
                            nc.tensor.matmul(
                                psl,
                                lhsT=ones_bf,
                                rhs=UT[:, jt - 1, :],
                                start=(jt == 1),
                                stop=False,
                            )
                            nc.tensor.matmul(
                                psl,
                                lhsT=ones_bf,
                                rhs=UT[:, jt, :],
                                start=False,
                                stop=(jt == NT - 1),
                            )
                    # pipeline: next chunk's E phase (early start for ACT)
                    if ic + 1 < NC_:
                        emit_sqrt(ic + 1)
                        emit_exp(ic + 1)
                    linv_row = et_pool.tile([1, SC], f32r, tag="et")
                    nc.vector.reciprocal(linv_row, psl)
                    # broadcast 1/l across partitions via K=1 matmul
                    psb = ps_d.tile([P, SC], f32, tag="psd")
                    nc.tensor.matmul(
                        psb, lhsT=ones_row, rhs=linv_row, start=True, stop=True
                    )
                    lbc = et_pool.tile([P, SC], f32, tag="et")
                    nc.vector.tensor_copy(lbc, psb)
                    # out^T[h, i] = V^T @ U^T, scaled by 1/l_i.
                    # Next chunk's d2+clamp+sqrt is interleaved per attnV group
                    # so DVE clamps never head-of-line block the outT multiplies
                    # and the d2 matmuls fill PE slot-wait gaps.
                    outT = ot_pool.tile([P, HT, SC], f32r, tag="outT")
                    for ht in range(HT):
                        pso = ps_o.tile([P, SC], f32, tag="pso")
                        for jt in range(NT):
                            nc.tensor.matmul(
                                pso,
                                lhsT=v_bf[:, jt, ht * P : (ht + 1) * P],
                                rhs=UT[:, jt, :],
                                start=(jt == 0),
                                stop=(jt == NT - 1),
                            )
                        nc.vector.tensor_mul(outT[:, ht, :], pso, lbc)
                    # final projection for this chunk's i-tiles; bias added
                    # in-place in PSUM, DMA reads PSUM directly
                    for it4 in range(ITC):
                        psy = ps_o.tile([P, SC], f32, tag="pso")
                        for ht in range(HT):
                            nc.tensor.matmul(
                                psy,
                                lhsT=outT[:, ht, it4 * P : (it4 + 1) * P],
                                rhs=wout_sb[:, ht, :],
                                start=(ht == 0),
                                stop=(ht == HT - 1),
                            )
                        ysb = small.tile([P, H], f32, tag="ysb")
                        nc.vector.tensor_add(ysb, psy, bo_bc)
                        nc.sync.dma_start(y_r[:, ic * ITC + it4, :], ysb)


def build_bass(n: int = 2048) -> bass.Bass:
    nc = bacc.Bacc(None, target_bir_lowering=False)
    x = nc.dram_tensor("x", [n, H], f32, kind="ExternalInput")[:, :]
    g = nc.dram_tensor("g", [n, 3], f32, kind="ExternalInput")[:, :]
    wqkv = nc.dram_tensor("w_qkv", [H, 3 * H], f32, kind="ExternalInput")[:, :]
    bqkv = nc.dram_tensor("b_qkv", [3 * H], f32, kind="ExternalInput")[:]
    wout = nc.dram_tensor("w_out", [H, H], f32, kind="ExternalInput")[:, :]
    bout = nc.dram_tensor("b_out", [H], f32, kind="ExternalInput")[:]
    y = nc.dram_tensor("y", [n, H], f32, kind="ExternalOutput")[:, :]
    with tile.TileContext(nc) as tc:
        _body(tc, n, x, g, wqkv, bqkv, wout, bout, y)
    nc.finalize()
    return nc


_CACHED = {}


def _get_nc(n: int = 2048) -> bass.Bass:
    if n not in _CACHED:
        _CACHED[n] = build_bass(n)
    return _CACHED[n]


def kernel(**inputs) -> np.ndarray:
    from concourse.bass_utils import run_bass_kernel_spmd

    x = np.ascontiguousarray(inputs["x"], dtype=np.float32)
    g = np.ascontiguousarray(inputs["geometric_features"], dtype=np.float32)
    wqkv = np.ascontiguousarray(inputs["W_qkv"], dtype=np.float32)
    bqkv = np.ascontiguousarray(inputs["b_qkv"], dtype=np.float32)
    wout = np.ascontiguousarray(inputs["W_out"], dtype=np.float32)
    bout = np.ascontiguousarray(inputs["b_out"], dtype=np.float32)

    B, n, _ = x.shape
    nc = _get_nc(n)
    core_ids = list(range(B))
    in_maps = [
        {
            "x": np.ascontiguousarray(x[b]),
            "g": np.ascontiguousarray(g[b]),
            "w_qkv": wqkv,
            "b_qkv": bqkv,
            "w_out": wout,
            "b_out": bout,
        }
        for b in range(B)
    ]
    res = run_bass_kernel_spmd(nc, in_maps, core_ids)
    return np.stack([res.results[b]["y"] for b in range(B)]).astype(np.float32)

